# revision 2
# baseline (speedup 1.0000x reference)
"""CRF decode (conv features -> emission scores -> Viterbi) on 8 TRN2 cores.

Data-parallel over the batch: each core gets B/8 = 4096 words (32 tiles of
128). The whole DP runs on a fixed-point grid (g = 2^-11) with the argmax
index packed into the low 5 fractional bits of the path value:

  T3[i,j] = round(T'/g) + (25-i)/32   -- emission/transition consts integer
  v       : integer-valued fp32 in (0, 2^18), so tot = v + T3 is exactly
            m + k/32 and the page max carries its own argmax.

Per step: per-word SEGMAX (custom DVE scan, stride-0 output keeps only the
page-final max) gives p* = m + k/32; Pool extracts r' = RNE(p*)-1 via the
+-2^23 trick, bp = p* - r' (positive, injective in k), v_new = r' + s_int,
and renormalizes v into the exactness window. Emissions are plain PE
matmuls on host-pre-transposed X (scores scaled by 1/g, rounded on Pool).
Backtrack selects bp rows via the EQSEL one-hot custom op.
"""

import sys

if "/opt/trn_rl_repo" not in sys.path:
    sys.path.insert(0, "/opt/trn_rl_repo")

import numpy as np

import copy as _copy
from dataclasses import dataclass
from typing import Any

import concourse.bacc as bacc
import concourse.mybir as mybir
import concourse.tile as tile
from concourse import bass_utils
from concourse import dve_ops
from concourse.dve_ops import DveOp
from concourse.dve_spec import Spec, Src0, Src1, lower, scan
from concourse.dve_uop import AluInp, AluOp as UAluOp, DveOpSpec, Trigger


_HAND_CACHE: dict = {}


@dataclass(frozen=True)
class _HandDveOp(DveOp):
    """DveOp whose compiled uops are post-processed by `patch` (adds the
    SUB_DIM_DONE step state that resets an in-body scan at page boundaries —
    not expressible in the Spec language)."""

    patch: Any = None

    def compile(self, ver):
        key = (self.name, ver)
        if (r := _HAND_CACHE.get(key)) is not None:
            return r
        uops = self.patch(lower(self.spec, ver=ver))
        for u in uops:
            u.validate(ver)
        res = DveOpSpec(
            name=self.name, opcode=dve_ops.get_dve_sub_opcode(self.name),
            uops=uops, rd1_en=True)
        _HAND_CACHE[key] = res
        return res


def _reset_scan_stage(dps):
    """In a copied steady datapath, turn the scan combine MAX(CURR, expr)
    into BYPASS(expr): the running max restarts from the current element."""
    for dp in dps:
        if (dp.op in (UAluOp.MAX, UAluOp.ADD)
                and dp.alu_src0 == AluInp.CURR_ALU_OUT):
            dp.op = UAluOp.BYPASS
            dp.alu_src0 = dp.alu_src1
            return True
    raise AssertionError("scan stage not found")


def _patch_segmax(uops):
    assert len(uops) == 2, uops
    seed, steady = uops
    steady.trigger = (Trigger.SRC_TENSOR_DONE, Trigger.SUB_DIM_DONE,
                      Trigger.NONE)
    steady.next_uop = (0, 2, 0)
    step = _copy.deepcopy(steady)
    step.trigger = (Trigger.SRC_TENSOR_DONE, Trigger.SUB_DIM_DONE,
                    Trigger.COUNT)
    step.next_uop = (0, 2, 1)
    step.repeat_count = 1
    _reset_scan_stage(step.datapath_config)
    return [seed, steady, step]


def _register_hand_op(name, spec, patch):
    if name in dve_ops._SUB_OPCODE_FOR_NAME:
        for op in dve_ops.OPS:
            if op.name == name:
                return op
    opcode = max(dve_ops._SUB_OPCODE_FOR_NAME.values()) + 1
    dve_ops._SUB_OPCODE_FOR_NAME[name] = opcode
    shas = {}
    for ver in ("v3", "v4"):
        uops = patch(lower(spec, ver=ver))
        s = DveOpSpec(name=name, opcode=opcode, uops=uops, rd1_en=True)
        shas[ver] = s.sha(ver)
    op = _HandDveOp(name, spec, True, shas, patch=patch)
    dve_ops.OPS.append(op)
    dve_ops.CUSTOM_DVE_SPECS[name] = spec
    return op


def _register_segmax():
    """r[p,s,n] = max over n' <= n (within page s) of (in0 + in1)[p,s,n'].
    The page-final slice [:, :, N-1] is the grouped max."""

    def _ref(in0, in1, s0, s1, imm2):
        N = in0.shape[-1]
        P = in0.shape[0]
        a = (np.asarray(in0, np.float32).reshape(P, -1, N)
             + np.asarray(in1, np.float32).reshape(P, -1, N))
        return np.maximum.accumulate(a, axis=2).reshape(in0.shape)

    spec = Spec(body=scan(UAluOp.MAX, Src0 + Src1), reference=_ref)
    return _register_hand_op("SEGMAX_ANT", spec, _patch_segmax)


SEGMAX = _register_segmax()


def _register_eqsel():
    """out[p,s,n] = (n == in1[p,s,n]) * in0[p,s,n] — one-hot select of a
    backpointer row by label index, one pass."""
    name = "EQSEL_ANT"
    if name in dve_ops._SUB_OPCODE_FOR_NAME:
        for op in dve_ops.OPS:
            if op.name == name:
                return op

    def _ref(in0, in1, s0, s1, imm2):
        N = in0.shape[-1]
        P = in0.shape[0]
        a = np.asarray(in0, np.float32).reshape(P, -1, N)
        b = np.asarray(in1, np.float32).reshape(a.shape)
        S = a.shape[1]
        n = (np.arange(S * N, dtype=np.float32)
             - np.repeat(np.arange(S), N) * s1).reshape(S, N)
        return ((n[None] == b).astype(np.float32) * a).reshape(in0.shape)

    from concourse.dve_spec import Idx, SubIdx, C1, eq
    spec = Spec(body=eq(Idx - SubIdx * C1, Src1) * Src0, reference=_ref)
    opcode = max(dve_ops._SUB_OPCODE_FOR_NAME.values()) + 1
    dve_ops._SUB_OPCODE_FOR_NAME[name] = opcode
    shas = {}
    for ver in ("v3", "v4"):
        sp = DveOpSpec(name=name, opcode=opcode, uops=lower(spec, ver=ver),
                       rd1_en=True)
        shas[ver] = sp.sha(ver)
    op = DveOp(name, spec, subdim=True, uops_sha=shas)
    dve_ops.OPS.append(op)
    dve_ops.CUSTOM_DVE_SPECS[name] = spec
    return op


EQSEL = _register_eqsel()





F32 = mybir.dt.float32
AX = mybir.AxisListType
OP = mybir.AluOpType
ACTF = mybir.ActivationFunctionType

B = 32768
M = 14
H, WD = 16, 8
F = 128
L = 26
KS = 5
NCORES = 8
BC = B // NCORES          # words per core (4096)
NT = BC // 128            # 128-word tiles per core (32)
GROUP_SIZES = [1, 1, 2, 4, 4, 4, 4, 4, 4, 4]
NG = len(GROUP_SIZES)
G = max(GROUP_SIZES)      # max tiles per group (for consts)
GL = G * L
GOFF = [sum(GROUP_SIZES[:i]) for i in range(NG)]

GBITS = 11
GQ = float(2.0 ** -GBITS)
OFFSET = float(2 ** 17)
RND = float(3 * 2 ** 22)   # 1.5*2^23: RNE magic for |x| < 2^22


def _conv_matrix(K: np.ndarray) -> np.ndarray:
    """C[o, i] such that conv_SAME(x.reshape(H,WD)) flattened == C @ x."""
    K2 = K.reshape(KS, KS).astype(np.float64)
    C = np.zeros((F, F), dtype=np.float64)
    for r in range(H):
        for c in range(WD):
            o = r * WD + c
            for dy in range(KS):
                for dx in range(KS):
                    rr = r + dy - KS // 2
                    cc = c + dx - KS // 2
                    if 0 <= rr < H and 0 <= cc < WD:
                        C[o, rr * WD + cc] = K2[dy, dx]
    return C


def _consts(K, b, W, T):
    """Host-side constant tensors (fp64 math, one final fp32 round)."""
    C = _conv_matrix(K)
    A = W.astype(np.float64) @ C                         # (L, F)
    c0 = float(b[0]) * W.astype(np.float64).sum(axis=1)  # (L,)
    Tp = T.astype(np.float64) + c0[None, :]              # T'[i,j]
    ATG = np.ascontiguousarray(A.T / GQ).astype(np.float32)   # (F, L)
    T3 = (np.rint(Tp / GQ)
          + ((25 - np.arange(L)) / 32.0)[:, None]).astype(np.float32)
    TTK3 = np.broadcast_to(
        np.ascontiguousarray(T3.T)[None], (128, L, L)).copy()
    # v0 = s0 + c0g + OFFSET, replicated per word-tile in group
    c0g = np.rint(c0 / GQ).astype(np.float32) + np.float32(OFFSET)
    C0OFFG = np.broadcast_to(np.tile(c0g, G)[None], (128, GL)).copy()
    finrow = ((25 - np.arange(L)) / 32.0).astype(np.float32)
    FINROWG = np.broadcast_to(np.tile(finrow, G)[None], (128, GL)).copy()
    return ATG, TTK3, C0OFFG, FINROWG


def build_module():
    nc = bacc.Bacc("TRN2", target_bir_lowering=False, debug=False,
                   num_devices=NCORES)
    xt_d = nc.dram_tensor("XT", [NT, F, M, 128], F32, kind="ExternalInput")
    atg_d = nc.dram_tensor("ATG", [F, L], F32, kind="ExternalInput")
    ttk_d = nc.dram_tensor("TTK3", [128, L, L], F32, kind="ExternalInput")
    c0_d = nc.dram_tensor("C0OFFG", [128, GL], F32, kind="ExternalInput")
    fin_d = nc.dram_tensor("FINROWG", [128, GL], F32, kind="ExternalInput")
    out_d = nc.dram_tensor("OUT", [BC, M], mybir.dt.int32,
                           kind="ExternalOutput")

    with tile.TileContext(nc) as tc:
        with (
            tc.tile_pool(name="const", bufs=1) as cpool,
            tc.tile_pool(name="pers", bufs=1) as ppool,
            tc.tile_pool(name="xwork", bufs=3) as xpool,
            tc.tile_pool(name="ework", bufs=4) as epool,
            tc.tile_pool(name="psum", bufs=2, space="PSUM") as psE,
        ):
            atg = cpool.tile([F, L], F32)
            ttk3 = cpool.tile([128, L, L], F32)
            c0offg = cpool.tile([128, GL], F32)
            finrowg = cpool.tile([128, GL], F32)
            nc.sync.dma_start(atg[:], atg_d.ap())
            nc.sync.dma_start(ttk3[:], ttk_d.ap())
            nc.sync.dma_start(c0offg[:], c0_d.ap())
            nc.sync.dma_start(finrowg[:], fin_d.ap())

            # persistent state
            s_all = ppool.tile([128, M, NT * L], F32)    # rounded s/g
            vall = ppool.tile([128, NT * L], F32)        # packed v (int fp32)
            pstar = ppool.tile([128, NT * L], F32)       # page maxes
            bp = ppool.tile([128, M - 1, NT * L], F32)   # frac'+1 per step
            path = ppool.tile([128, NT, M], F32)
            pf = ppool.tile([128, NT], F32)              # final packed max
            lab = ppool.tile([128, NT], F32)             # current label

            # ---- emissions: per tile DMA + 14 matmuls + ACT copy ----
            for n in range(NT):
                xt = xpool.tile([F, M * 128], F32, tag="xt")
                nc.sync.dma_start(xt[:], xt_d.ap()[n])
                ps = psE.tile([128, M * L], F32, tag="eps")
                for m in range(M):
                    nc.tensor.matmul(ps[:, m * L:(m + 1) * L],
                                     xt[:, m * 128:(m + 1) * 128], atg[:])
                nc.scalar.activation(
                    s_all[:, :, n * L:(n + 1) * L],
                    ps[:].rearrange("p (m l) -> p m l", l=L), ACTF.Copy)

            # ---- forward DP: diagonal wavefront over (group, t) ----
            for w in range(1, NG + M - 1):
                for gi in range(NG):
                    t = w - gi
                    if t < 1 or t > M - 1:
                        continue
                    gg = GROUP_SIZES[gi]
                    gl = gg * L
                    sl = slice(GOFF[gi] * L, GOFF[gi] * L + gl)
                    if t == 1:
                        # round s0, init v0 = s0 + c0g + OFFSET (Pool)
                        nc.gpsimd.tensor_scalar(
                            s_all[:, 0, sl], s_all[:, 0, sl], RND, RND,
                            op0=OP.add, op1=OP.subtract)
                        nc.gpsimd.tensor_tensor(
                            vall[:, sl], s_all[:, 0, sl], c0offg[:, :gl],
                            op=OP.add)
                    # per-word SEGMAX with stride-0 (compact) output
                    for k in range(gg):
                        o = GOFF[gi] * L + k * L
                        nc.vector._custom_dve(
                            SEGMAX,
                            out=pstar[:, o:o + L].unsqueeze(2)
                                .broadcast_to((128, L, L)),
                            in0=ttk3[:],
                            in1=vall[:, o:o + L].unsqueeze(1)
                                .broadcast_to((128, L, L)))
                    # round this step's s, then extraction (Pool)
                    nc.gpsimd.tensor_scalar(
                        s_all[:, t, sl], s_all[:, t, sl], RND, RND,
                        op0=OP.add, op1=OP.subtract)
                    rp = epool.tile([128, GL], F32, tag="rp")
                    nc.gpsimd.tensor_scalar(
                        rp[:, :gl], pstar[:, sl], RND, RND + 1.0,
                        op0=OP.add, op1=OP.subtract)      # r' = RNE(p*) - 1
                    nc.gpsimd.tensor_tensor(
                        bp[:, t - 1, sl], pstar[:, sl], rp[:, :gl],
                        op=OP.subtract)                   # frac' + 1 > 0
                    vn = epool.tile([128, GL], F32, tag="vn")
                    nc.gpsimd.tensor_tensor(
                        vn[:, :gl], rp[:, :gl], s_all[:, t, sl], op=OP.add)
                    for k in range(gg):
                        o = k * L
                        nc.gpsimd.tensor_tensor(
                            vall[:, GOFF[gi] * L + o:GOFF[gi] * L + o + L],
                            vn[:, o:o + L],
                            vn[:, o:o + 1].broadcast_to((128, L)),
                            op=OP.subtract)
                    nc.gpsimd.tensor_scalar(
                        vall[:, sl], vall[:, sl], OFFSET, None, op0=OP.add)

            # ---- final label: pack (25-j)/32, page max, extract ----
            for gi in range(NG):
                gg = GROUP_SIZES[gi]
                gl = gg * L
                sl = slice(GOFF[gi] * L, GOFF[gi] * L + gl)
                vq = epool.tile([128, GL], F32, tag="vq")
                nc.gpsimd.tensor_tensor(vq[:, :gl], vall[:, sl],
                                        finrowg[:, :gl], op=OP.add)
                nc.vector.tensor_reduce(
                    pf[:, GOFF[gi]:GOFF[gi] + gg],
                    vq[:, :gl].rearrange("p (k l) -> p k l", l=L),
                    axis=AX.X, op=OP.max)
            rf = ppool.tile([128, NT], F32)
            nc.vector.tensor_scalar(rf[:], pf[:], RND, RND + 1.0,
                                    op0=OP.add, op1=OP.subtract)
            frv = ppool.tile([128, NT], F32)
            nc.vector.tensor_tensor(frv[:], pf[:], rf[:], op=OP.subtract)
            # label = (57 - 32*bp') mod 32-ish: ip = -32*bp' + 57; -32 if >=26
            ipv = ppool.tile([128, NT], F32)
            nc.vector.tensor_scalar(ipv[:], frv[:], -32.0, 57.0,
                                    op0=OP.mult, op1=OP.add)
            m1 = ppool.tile([128, NT], F32)
            nc.vector.tensor_scalar(m1[:], ipv[:], 26.0, None, op0=OP.is_ge)
            nc.vector.scalar_tensor_tensor(
                path[:, :, M - 1], m1[:], -32.0, ipv[:],
                op0=OP.mult, op1=OP.add)

            # ---- backtrack (all-DVE chain) ----
            for t in range(M - 2, -1, -1):
                ew = epool.tile([128, NT, L], F32, tag="ew")
                nc.vector._custom_dve(
                    EQSEL, out=ew[:],
                    in0=bp[:, t, :].rearrange("p (k l) -> p k l", l=L),
                    in1=path[:, :, t + 1].unsqueeze(2)
                        .broadcast_to((128, NT, L)),
                    s1=float(L))
                fsel = epool.tile([128, NT], F32, tag="fsel")
                nc.vector.tensor_reduce(fsel[:], ew[:], axis=AX.X, op=OP.max)
                nc.vector.tensor_scalar(ipv[:], fsel[:], -32.0, 57.0,
                                        op0=OP.mult, op1=OP.add)
                nc.vector.tensor_scalar(m1[:], ipv[:], 26.0, None,
                                        op0=OP.is_ge)
                nc.vector.scalar_tensor_tensor(
                    path[:, :, t], m1[:], -32.0, ipv[:],
                    op0=OP.mult, op1=OP.add)

            pi = ppool.tile([128, NT, M], mybir.dt.int32)
            nc.vector.tensor_copy(pi[:], path[:])
            out_t = out_d.ap().rearrange("(n p) m -> p n m", p=128)
            nc.sync.dma_start(out_t, pi[:])

    nc.compile()
    return nc


_CACHE = {}


def _get_module():
    if "nc" not in _CACHE:
        _CACHE["nc"] = build_module()
    return _CACHE["nc"]


def make_in_maps(X, K, b, W, T):
    ATG, TTK3, C0OFFG, FINROWG = _consts(K, b, W, T)
    consts = {"ATG": ATG, "TTK3": TTK3, "C0OFFG": C0OFFG, "FINROWG": FINROWG}
    X = np.ascontiguousarray(X, dtype=np.float32)
    # per-core pre-transposed layout XT[n, f, m, w] = X[c*BC + n*128 + w, m, f]
    X5 = X.reshape(NCORES, NT, 128, M, F)
    in_maps = []
    for c in range(NCORES):
        xt = np.ascontiguousarray(X5[c].transpose(0, 3, 2, 1))
        in_maps.append(dict(consts, XT=xt))
    return in_maps


def kernel(X, K, b, W, T):
    nc = _get_module()
    in_maps = make_in_maps(X, K, b, W, T)
    res = bass_utils.run_bass_kernel_spmd(nc, in_maps,
                                          core_ids=list(range(NCORES)))
    out = np.concatenate([res.results[c]["OUT"] for c in range(NCORES)],
                         axis=0)
    return out.reshape(B, M, 1).astype(np.int32)
